# revision 13
# baseline (speedup 1.0000x reference)
"""Trainium2 Bass kernel for nn_Attention_65317862637882.

Data-parallel over batch (B=32) on 8 NeuronCores (4 batches/core).

Math (per batch, O=1):
  q      = W_in @ query                      [1024]
  scores = context @ q                       [2048]
  attn   = softmax(scores)
  bt     = exp(-ab * delta_t),  delta_t[l] = L-1-l
  mix    = attn * context^T                  [1024, 2048]
  term2  = relu(ae * mix * bt)
  mixed  = sum_l(term2 + mix)                [1024]
  out    = tanh(W_out @ [mixed; q])          [512]

Key reformulation: ae, attn, bt >= 0, so
  sum_l relu(ae*attn[l]*bt[l]*C[l,d]) = sum_l w2[l]*relu(C[l,d]),
  w2 = ae*attn*bt.  Hence
  mixed[d] = sum_l attn[l]*C[l,d] + sum_l w2[l]*relu(C[l,d])
i.e. two vector-stationary GEMVs contracting l (the SBUF partition dim of
naturally-laid-out context tiles) on the TensorEngine, plus one relu pass
(ScalarE).  scores needs the d-contraction, which the PE cannot do in this
layout; it runs on VectorE as a fused multiply+reduce (tensor_tensor_reduce)
against a broadcast copy of q.  Context is read from HBM exactly once.

Per-core engine budget (4 batches): DMA ~38MB (bound), DVE ~78us,
ACT ~68us, PE ~50us, GpSimd small.
"""

import os
import sys

import numpy as np

_HERE = os.path.dirname(os.path.abspath(__file__))
if _HERE not in sys.path:
    sys.path.insert(0, _HERE)

import concourse.bacc as bacc
import concourse.bass as bass  # noqa: F401
import concourse.mybir as mybir
import concourse.tile as tile
from concourse.bass_isa import ReduceOp  # noqa: E402
from concourse.bass_utils import run_bass_kernel_spmd

try:
    from tile_patch import legalize_single_wait
except ImportError:
    # kernel.py must be self-contained: inline fallback of the walrus
    # single-sync-wait legalization (see tile_patch.py).
    _uid = [0]

    def legalize_single_wait(nc):
        for fn in nc.m.functions:
            for bb in fn.blocks:
                new_insts = []
                changed = False
                for inst in bb.instructions:
                    si = inst.sync_info
                    if si is not None and len(si.on_wait) > 1:
                        waits = list(si.on_wait)
                        for w in waits[:-1]:
                            _uid[0] += 1
                            nop = mybir.InstNoOp(
                                name=f"I-swl-{_uid[0]}",
                                engine=inst.engine,
                                sync_info=mybir.SyncInfo(on_wait=[w], on_update=[]),
                            )
                            new_insts.append(nop)
                        si.on_wait = waits[-1:]
                        changed = True
                    new_insts.append(inst)
                if changed:
                    bb.instructions = new_insts


NCORES = 8
B, L, D, D2 = 32, 2048, 1024, 512
BPC = B // NCORES            # batches per core
P = 128                      # partitions
NLC = L // P                 # 16 l-chunks
NDC = D // P                 # 8 d-chunks
NCC = 2 * D // P             # 16 c-chunks of combined
NOT = D2 // P                # 4 o-tiles

F32 = mybir.dt.float32
BF16 = mybir.dt.bfloat16
AF = mybir.ActivationFunctionType
ALU = mybir.AluOpType
AX = mybir.AxisListType


def build_bass():
    nc = bacc.Bacc("TRN2", target_bir_lowering=False)

    qt_d = nc.declare_dram_parameter("qt", [P, NDC * BPC], F32, isOutput=False)          # [p,(dc,b)]
    ctx_d = nc.declare_dram_parameter("ctx", [BPC, NLC, P, D], F32, isOutput=False)
    wint_d = nc.declare_dram_parameter("wint", [NDC, P, D], F32, isOutput=False)         # W_in.T
    woutt_d = nc.declare_dram_parameter("woutt", [NCC, P, D2], BF16, isOutput=False)    # W_out.T bf16
    w2b_d = nc.declare_dram_parameter("w2b", [BPC, P, NLC], F32, isOutput=False)         # ae*exp(-ab*delta)
    out_d = nc.declare_dram_parameter("out_o", [P, NOT * BPC], F32, isOutput=True)
    attn_d = nc.declare_dram_parameter("attn_o", [BPC, P, NLC], F32, isOutput=True)

    with tile.TileContext(nc) as tc:
        with (
            tc.tile_pool(name="wts", bufs=3) as kpool,
            tc.tile_pool(name="singles", bufs=1) as qpool,
            tc.tile_pool(name="cdata", bufs=8) as cpool,
            tc.tile_pool(name="rdata", bufs=20) as rpool,
            tc.tile_pool(name="cbdata", bufs=20) as cbpool,
            tc.tile_pool(name="qb", bufs=2) as qbpool,
            tc.tile_pool(name="smalls", bufs=6) as spool,
            tc.tile_pool(name="psA", bufs=1, space="PSUM") as psa,
            tc.tile_pool(name="dramb", bufs=2, space="DRAM") as dpool,
            tc.tile_pool(name="psB", bufs=2, space="PSUM") as psb,
        ):
            # ---- W_in GEMM, both output layouts --------------------------
            qt = qpool.tile([P, NDC * BPC], F32)
            nc.sync.dma_start(out=qt[:], in_=qt_d[:])

            qrow_ps = psa.tile([BPC, D], F32)      # q as rows (for broadcast)
            for dc in range(NDC):
                w = kpool.tile([P, D], F32)
                nc.sync.dma_start(out=w[:], in_=wint_d[dc])
                lhs_q = qt[:, dc * BPC:(dc + 1) * BPC]
                nc.tensor.matmul(qrow_ps[:, 0:512], lhs_q, w[:, 0:512],
                                 start=(dc == 0), stop=(dc == NDC - 1))
                nc.tensor.matmul(qrow_ps[:, 512:1024], lhs_q, w[:, 512:1024],
                                 start=(dc == 0), stop=(dc == NDC - 1))

            q_rows = qpool.tile([BPC, D], F32)
            nc.vector.tensor_copy(q_rows[:], qrow_ps[:])

            w2b_sb = qpool.tile([P, BPC * NLC], F32)
            for b in range(BPC):
                nc.scalar.dma_start(out=w2b_sb[:, b * NLC:(b + 1) * NLC],
                                    in_=w2b_d[b])

            comb = qpool.tile([P, NCC, BPC], F32)   # [c-part,(cc,b)]
            ones1 = qpool.tile([P, 1], F32)
            nc.vector.memset(ones1[:], 1.0)
            warm_ps = psa.tile([1, P], F32)

            # ---- per-batch main pipeline ---------------------------------
            for b in range(BPC):
                qrow_b = spool.tile([1, D], F32)
                nc.scalar.dma_start(out=qrow_b[:], in_=q_rows[b:b + 1, :])
                dq = dpool.tile([1, D], F32, tag="dq")
                nc.scalar.dma_start(out=dq[:], in_=qrow_b[:])
                nc.scalar.dma_start(
                    out=comb[:, NDC:NCC, b],
                    in_=dq[:].rearrange("a (c p) -> a p c", c=NDC).squeeze(0),
                )
                qb = qbpool.tile([P, D], F32)
                nc.gpsimd.partition_broadcast(qb[:], qrow_b[:])

                scores = spool.tile([P, NLC], F32)
                cb_tiles = []
                r_tiles = []
                for i in range(NLC):
                    ct = cpool.tile([P, D], F32)
                    nc.sync.dma_start(out=ct[:], in_=ctx_d[b, i])
                    rt = rpool.tile([P, D], BF16)
                    nc.scalar.activation(rt[:], ct[:], AF.Relu)
                    cbt = cbpool.tile([P, D], BF16)
                    nc.gpsimd.tensor_copy(cbt[:], ct[:])
                    scr = qbpool.tile([P, D], F32, tag="amr_scr")
                    nc.vector.affine_mul_reduce(
                        scr[:], scores[:, i:i + 1], ct[:], qb[:], 1.0, 0.0,
                    )
                    # keep the PE HAM-warm: one cheap matmul per arriving chunk
                    nc.tensor.matmul(warm_ps[:], ones1[:], ct[:, 0:P],
                                     start=True, stop=True)
                    cb_tiles.append(cbt)
                    r_tiles.append(rt)

                # softmax over all 2048 scores (layout [128, 16])
                mx = spool.tile([P, 1], F32)
                nc.vector.tensor_reduce(mx[:], scores[:], axis=AX.X, op=ALU.max)
                nc.gpsimd.partition_all_reduce(mx[:], mx[:], P, ReduceOp.max)
                negmx = spool.tile([P, 1], F32)
                nc.vector.tensor_scalar_mul(negmx[:], mx[:], -1.0)
                ex = spool.tile([P, NLC], F32)
                nc.scalar.activation(ex[:], scores[:], AF.Exp, bias=negmx[:])
                sm = spool.tile([P, 1], F32)
                nc.vector.tensor_reduce(sm[:], ex[:], axis=AX.X, op=ALU.add)
                nc.gpsimd.partition_all_reduce(sm[:], sm[:], P, ReduceOp.add)
                rinv = spool.tile([P, 1], F32)
                nc.vector.reciprocal(rinv[:], sm[:])
                attn = spool.tile([P, NLC], F32)
                nc.vector.tensor_scalar_mul(attn[:], ex[:], rinv[:])
                nc.scalar.dma_start(out=attn_d[b], in_=attn[:])

                w2 = spool.tile([P, NLC], F32)
                nc.vector.tensor_mul(w2[:], attn[:],
                                     w2b_sb[:, b * NLC:(b + 1) * NLC])
                w2bf = spool.tile([P, NLC], BF16)
                nc.vector.tensor_copy(w2bf[:], w2[:])
                attn_bf = spool.tile([P, NLC], BF16)
                nc.vector.tensor_copy(attn_bf[:], attn[:])

                # mixed[d] = sum_l attn[l]*C[l,d] + w2[l]*relu(C)[l,d]
                # moving-operand GEMV: vector stationary (1-col LDW), C/R
                # tiles stream through the PE; out rows [1, 1024] in PSUM.
                mixed_ps = psb.tile([1, D], F32)
                for h in range(2):          # two 512-col banks
                    sl = slice(h * 512, (h + 1) * 512)
                    for i in range(NLC):
                        nc.tensor.matmul(
                            mixed_ps[:, sl],
                            attn_bf[:, i:i + 1],
                            cb_tiles[i][:, sl],
                            start=(i == 0), stop=False,
                        )
                    for i in range(NLC):
                        nc.tensor.matmul(
                            mixed_ps[:, sl],
                            w2bf[:, i:i + 1],
                            r_tiles[i][:, sl],
                            start=False, stop=(i == NLC - 1),
                        )
                # bounce row -> [128 d-part, 8 chunks] into comb via DRAM
                mrow = spool.tile([1, D], F32)
                nc.vector.tensor_copy(mrow[:], mixed_ps[:])
                dmix = dpool.tile([1, D], F32)
                nc.scalar.dma_start(out=dmix[:], in_=mrow[:])
                nc.scalar.dma_start(
                    out=comb[:, 0:NDC, b],
                    in_=dmix[:].rearrange("a (c p) -> a p c", c=NDC).squeeze(0),
                )

            # ---- W_out GEMM (batched) + tanh -----------------------------
            comb_bf = qpool.tile([P, NCC, BPC], BF16)
            nc.vector.tensor_copy(comb_bf[:], comb[:])
            ops_ps = psa.tile([P, NOT * BPC], F32)
            for cc in range(NCC):
                wo = kpool.tile([P, D2], BF16)
                nc.sync.dma_start(out=wo[:], in_=woutt_d[cc])
                for ot in range(NOT):
                    nc.tensor.matmul(
                        ops_ps[:, ot * BPC:(ot + 1) * BPC],
                        wo[:, ot * P:(ot + 1) * P],
                        comb_bf[:, cc, :],
                        start=(cc == 0 and ot == 0), stop=(cc == NCC - 1),
                    )
            out_sb = qpool.tile([P, NOT * BPC], F32)
            nc.scalar.activation(out_sb[:], ops_ps[:], AF.Tanh)
            nc.scalar.dma_start(out=out_d[:], in_=out_sb[:])

    nc.compile()
    legalize_single_wait(nc)
    return nc


_NC_CACHE = None


def _get_nc():
    global _NC_CACHE
    if _NC_CACHE is None:
        _NC_CACHE = build_bass()
    return _NC_CACHE


def _prep_in_maps(query, context, W_in, W_out, ae, ab):
    query = np.asarray(query, dtype=np.float32)
    context = np.asarray(context, dtype=np.float32)
    W_in = np.asarray(W_in, dtype=np.float32)
    W_out = np.asarray(W_out, dtype=np.float32)
    ae = np.asarray(ae, dtype=np.float32).reshape(B)
    ab = np.asarray(ab, dtype=np.float32).reshape(B)

    # Replicated weights, host-transposed for natural SBUF tiling.
    wint = np.ascontiguousarray(W_in.T.reshape(NDC, P, D))
    import ml_dtypes
    woutt = np.ascontiguousarray(
        W_out.T.reshape(NCC, P, D2).astype(ml_dtypes.bfloat16)
    )

    # delta in [p, c] layout: delta[c*128+p] = L-1-(c*128+p)
    lidx = (np.arange(NLC)[None, :] * P + np.arange(P)[:, None]).astype(np.float32)
    delta_pc = (L - 1) - lidx                                    # [128, 16]

    in_maps = []
    for m in range(NCORES):
        bs = slice(m * BPC, (m + 1) * BPC)
        q = query[bs, 0, :]                                       # [4, 1024]
        qt = np.ascontiguousarray(
            q.T.reshape(NDC, P, BPC).transpose(1, 0, 2).reshape(P, NDC * BPC)
        )
        ctx = np.ascontiguousarray(context[bs].reshape(BPC, NLC, P, D))
        ae_m = ae[bs]
        ab_m = ab[bs]
        w2b = (ae_m[:, None, None]
               * np.exp(-ab_m[:, None, None] * delta_pc[None])).astype(np.float32)
        in_maps.append({
            "qt": qt,
            "ctx": ctx,
            "wint": wint,
            "woutt": woutt,
            "w2b": np.ascontiguousarray(w2b),
        })
    return in_maps


def _assemble(results):
    out = np.empty((B, 1, D2), dtype=np.float32)
    attn = np.empty((B, 1, L), dtype=np.float32)
    for m in range(NCORES):
        o = results[m]["out_o"]                       # [128, 16] = [p,(ot,b)]
        a = results[m]["attn_o"]                      # [4, 128, 16] = [b,p,c]
        out[m * BPC:(m + 1) * BPC, 0, :] = (
            o.reshape(P, NOT, BPC).transpose(2, 1, 0).reshape(BPC, D2)
        )
        attn[m * BPC:(m + 1) * BPC, 0, :] = (
            a.transpose(0, 2, 1).reshape(BPC, L)
        )
    return out, attn


def kernel(query, context, W_in, W_out, ae, ab):
    nc = _get_nc()
    in_maps = _prep_in_maps(query, context, W_in, W_out, ae, ab)
    res = run_bass_kernel_spmd(nc, in_maps, core_ids=list(range(NCORES)))
    return _assemble(res.results)


if __name__ == "__main__":
    # quick self-run against reference
    import reference

    inputs = {k: np.asarray(v) for k, v in reference.setup_inputs().items()}
    got_out, got_attn = kernel(**inputs)
    exp_out, exp_attn = reference.reference(**reference.setup_inputs())
    exp_out, exp_attn = np.asarray(exp_out), np.asarray(exp_attn)
    for name, g, e in (("out", got_out, exp_out), ("attn", got_attn, exp_attn)):
        rel = np.linalg.norm(g - e) / np.linalg.norm(e)
        print(f"{name}: rel_err={rel:.3e} max={np.abs(g - e).max():.3e}")


# revision 14
# speedup vs baseline: 1.2991x; 1.2991x over previous
"""Trainium2 Bass kernel for nn_Attention_65317862637882.

Data-parallel over batch (B=32) on 8 NeuronCores (4 batches/core).

Math (per batch, O=1):
  q      = W_in @ query                      [1024]
  scores = context @ q                       [2048]
  attn   = softmax(scores)
  bt     = exp(-ab * delta_t),  delta_t[l] = L-1-l
  mix    = attn * context^T                  [1024, 2048]
  term2  = relu(ae * mix * bt)
  mixed  = sum_l(term2 + mix)                [1024]
  out    = tanh(W_out @ [mixed; q])          [512]

Key reformulation: ae, attn, bt >= 0, so
  sum_l relu(ae*attn[l]*bt[l]*C[l,d]) = sum_l w2[l]*relu(C[l,d]),
  w2 = ae*attn*bt.  Hence
  mixed[d] = sum_l attn[l]*C[l,d] + sum_l w2[l]*relu(C[l,d])
i.e. two vector-stationary GEMVs contracting l (the SBUF partition dim of
naturally-laid-out context tiles) on the TensorEngine, plus one relu pass
(ScalarE).  scores needs the d-contraction, which the PE cannot do in this
layout; it runs on VectorE as a fused multiply+reduce (tensor_tensor_reduce)
against a broadcast copy of q.  Context is read from HBM exactly once.

Per-core engine budget (4 batches): DMA ~38MB (bound), DVE ~78us,
ACT ~68us, PE ~50us, GpSimd small.
"""

import os
import sys

import numpy as np

_HERE = os.path.dirname(os.path.abspath(__file__))
if _HERE not in sys.path:
    sys.path.insert(0, _HERE)

import concourse.bacc as bacc
import concourse.bass as bass  # noqa: F401
import concourse.mybir as mybir
import concourse.tile as tile
from concourse.bass_isa import ReduceOp  # noqa: E402
from concourse.bass_utils import run_bass_kernel_spmd

try:
    from tile_patch import legalize_single_wait
except ImportError:
    # kernel.py must be self-contained: inline fallback of the walrus
    # single-sync-wait legalization (see tile_patch.py).
    _uid = [0]

    def legalize_single_wait(nc):
        for fn in nc.m.functions:
            for bb in fn.blocks:
                new_insts = []
                changed = False
                for inst in bb.instructions:
                    si = inst.sync_info
                    if si is not None and len(si.on_wait) > 1:
                        waits = list(si.on_wait)
                        for w in waits[:-1]:
                            _uid[0] += 1
                            nop = mybir.InstNoOp(
                                name=f"I-swl-{_uid[0]}",
                                engine=inst.engine,
                                sync_info=mybir.SyncInfo(on_wait=[w], on_update=[]),
                            )
                            new_insts.append(nop)
                        si.on_wait = waits[-1:]
                        changed = True
                    new_insts.append(inst)
                if changed:
                    bb.instructions = new_insts


NCORES = 8
B, L, D, D2 = 32, 2048, 1024, 512
BPC = B // NCORES            # batches per core
P = 128                      # partitions
NLC = L // P                 # 16 l-chunks
NDC = D // P                 # 8 d-chunks
NCC = 2 * D // P             # 16 c-chunks of combined
NOT = D2 // P                # 4 o-tiles

F32 = mybir.dt.float32
BF16 = mybir.dt.bfloat16
AF = mybir.ActivationFunctionType
ALU = mybir.AluOpType
AX = mybir.AxisListType


def build_bass():
    nc = bacc.Bacc("TRN2", target_bir_lowering=False)

    qt_d = nc.declare_dram_parameter("qt", [P, NDC * BPC], F32, isOutput=False)          # [p,(dc,b)]
    ctx_d = nc.declare_dram_parameter("ctx", [BPC, NLC, P, D], F32, isOutput=False)
    wint_d = nc.declare_dram_parameter("wint", [NDC, P, D], F32, isOutput=False)         # W_in.T
    woutt_d = nc.declare_dram_parameter("woutt", [NCC, P, D2], BF16, isOutput=False)    # W_out.T bf16
    w2b_d = nc.declare_dram_parameter("w2b", [BPC, P, NLC], F32, isOutput=False)         # ae*exp(-ab*delta)
    out_d = nc.declare_dram_parameter("out_o", [P, NOT * BPC], F32, isOutput=True)
    attn_d = nc.declare_dram_parameter("attn_o", [BPC, P, NLC], F32, isOutput=True)

    with tile.TileContext(nc) as tc:
        with (
            tc.tile_pool(name="wts", bufs=3) as kpool,
            tc.tile_pool(name="singles", bufs=1) as qpool,
            tc.tile_pool(name="cdata", bufs=20) as cpool,
            tc.tile_pool(name="rdata", bufs=20) as rpool,
            tc.tile_pool(name="qb", bufs=2) as qbpool,
            tc.tile_pool(name="smalls", bufs=6) as spool,
            tc.tile_pool(name="psA", bufs=1, space="PSUM") as psa,
            tc.tile_pool(name="dramb", bufs=2, space="DRAM") as dpool,
            tc.tile_pool(name="psB", bufs=2, space="PSUM") as psb,
        ):
            # ---- W_in GEMM, both output layouts --------------------------
            qt = qpool.tile([P, NDC * BPC], F32)
            nc.sync.dma_start(out=qt[:], in_=qt_d[:])

            qrow_ps = psa.tile([BPC, D], F32)      # q as rows (for broadcast)
            for dc in range(NDC):
                w = kpool.tile([P, D], F32)
                nc.sync.dma_start(out=w[:], in_=wint_d[dc])
                lhs_q = qt[:, dc * BPC:(dc + 1) * BPC]
                nc.tensor.matmul(qrow_ps[:, 0:512], lhs_q, w[:, 0:512],
                                 start=(dc == 0), stop=(dc == NDC - 1))
                nc.tensor.matmul(qrow_ps[:, 512:1024], lhs_q, w[:, 512:1024],
                                 start=(dc == 0), stop=(dc == NDC - 1))

            q_rows = qpool.tile([BPC, D], F32)
            nc.vector.tensor_copy(q_rows[:], qrow_ps[:])

            w2b_sb = qpool.tile([P, BPC * NLC], F32)
            for b in range(BPC):
                nc.scalar.dma_start(out=w2b_sb[:, b * NLC:(b + 1) * NLC],
                                    in_=w2b_d[b])

            comb = qpool.tile([P, NCC, BPC], F32)   # [c-part,(cc,b)]
            ones1 = qpool.tile([P, 1], F32)
            nc.vector.memset(ones1[:], 1.0)
            warm_ps = psa.tile([1, P], F32)

            # ---- per-batch main pipeline ---------------------------------
            for b in range(BPC):
                qrow_b = spool.tile([1, D], F32)
                nc.scalar.dma_start(out=qrow_b[:], in_=q_rows[b:b + 1, :])
                dq = dpool.tile([1, D], F32, tag="dq")
                nc.scalar.dma_start(out=dq[:], in_=qrow_b[:])
                nc.scalar.dma_start(
                    out=comb[:, NDC:NCC, b],
                    in_=dq[:].rearrange("a (c p) -> a p c", c=NDC).squeeze(0),
                )
                qb = qbpool.tile([P, D], F32)
                nc.gpsimd.partition_broadcast(qb[:], qrow_b[:])

                scores = spool.tile([P, NLC], F32)
                c_tiles = []
                r_tiles = []
                for i in range(NLC):
                    ct = cpool.tile([P, D], F32)
                    nc.sync.dma_start(out=ct[:], in_=ctx_d[b, i])
                    rt = rpool.tile([P, D], BF16)
                    nc.scalar.activation(rt[:], ct[:], AF.Relu)
                    scr = qbpool.tile([P, D], F32, tag="amr_scr")
                    nc.vector.affine_mul_reduce(
                        scr[:], scores[:, i:i + 1], ct[:], qb[:], 1.0, 0.0,
                    )
                    # keep the PE HAM-warm: one cheap matmul per arriving chunk
                    nc.tensor.matmul(warm_ps[:], ones1[:], ct[:, 0:P],
                                     start=True, stop=True)
                    c_tiles.append(ct)
                    r_tiles.append(rt)

                # softmax over all 2048 scores (layout [128, 16])
                mx = spool.tile([P, 1], F32)
                nc.vector.tensor_reduce(mx[:], scores[:], axis=AX.X, op=ALU.max)
                nc.gpsimd.partition_all_reduce(mx[:], mx[:], P, ReduceOp.max)
                negmx = spool.tile([P, 1], F32)
                nc.vector.tensor_scalar_mul(negmx[:], mx[:], -1.0)
                ex = spool.tile([P, NLC], F32)
                nc.scalar.activation(ex[:], scores[:], AF.Exp, bias=negmx[:])
                sm = spool.tile([P, 1], F32)
                nc.vector.tensor_reduce(sm[:], ex[:], axis=AX.X, op=ALU.add)
                nc.gpsimd.partition_all_reduce(sm[:], sm[:], P, ReduceOp.add)
                rinv = spool.tile([P, 1], F32)
                nc.vector.reciprocal(rinv[:], sm[:])
                attn = spool.tile([P, NLC], F32)
                nc.vector.tensor_scalar_mul(attn[:], ex[:], rinv[:])
                nc.scalar.dma_start(out=attn_d[b], in_=attn[:])

                w2 = spool.tile([P, NLC], F32)
                nc.vector.tensor_mul(w2[:], attn[:],
                                     w2b_sb[:, b * NLC:(b + 1) * NLC])
                w2bf = spool.tile([P, NLC], BF16)
                nc.vector.tensor_copy(w2bf[:], w2[:])

                # mixed[d] = sum_l attn[l]*C[l,d] + w2[l]*relu(C)[l,d]
                # moving-operand GEMV: vector stationary (1-col LDW), C/R
                # tiles stream through the PE; out rows [1, 1024] in PSUM.
                mixed_ps = psb.tile([1, D], F32)
                for h in range(2):          # two 512-col banks
                    sl = slice(h * 512, (h + 1) * 512)
                    for i in range(NLC):
                        nc.tensor.matmul(
                            mixed_ps[:, sl],
                            attn[:, i:i + 1],
                            c_tiles[i][:, sl],
                            start=(i == 0), stop=False,
                        )
                    for i in range(NLC):
                        nc.tensor.matmul(
                            mixed_ps[:, sl],
                            w2bf[:, i:i + 1],
                            r_tiles[i][:, sl],
                            start=False, stop=(i == NLC - 1),
                        )
                # bounce row -> [128 d-part, 8 chunks] into comb via DRAM
                mrow = spool.tile([1, D], F32)
                nc.vector.tensor_copy(mrow[:], mixed_ps[:])
                dmix = dpool.tile([1, D], F32)
                nc.scalar.dma_start(out=dmix[:], in_=mrow[:])
                nc.scalar.dma_start(
                    out=comb[:, 0:NDC, b],
                    in_=dmix[:].rearrange("a (c p) -> a p c", c=NDC).squeeze(0),
                )

            # ---- W_out GEMM (batched) + tanh -----------------------------
            comb_bf = qpool.tile([P, NCC, BPC], BF16)
            nc.vector.tensor_copy(comb_bf[:], comb[:])
            ops_ps = psa.tile([P, NOT * BPC], F32)
            for cc in range(NCC):
                wo = kpool.tile([P, D2], BF16)
                nc.sync.dma_start(out=wo[:], in_=woutt_d[cc])
                for ot in range(NOT):
                    nc.tensor.matmul(
                        ops_ps[:, ot * BPC:(ot + 1) * BPC],
                        wo[:, ot * P:(ot + 1) * P],
                        comb_bf[:, cc, :],
                        start=(cc == 0 and ot == 0), stop=(cc == NCC - 1),
                    )
            out_sb = qpool.tile([P, NOT * BPC], F32)
            nc.scalar.activation(out_sb[:], ops_ps[:], AF.Tanh)
            nc.scalar.dma_start(out=out_d[:], in_=out_sb[:])

    nc.compile()
    legalize_single_wait(nc)
    return nc


_NC_CACHE = None


def _get_nc():
    global _NC_CACHE
    if _NC_CACHE is None:
        _NC_CACHE = build_bass()
    return _NC_CACHE


def _prep_in_maps(query, context, W_in, W_out, ae, ab):
    query = np.asarray(query, dtype=np.float32)
    context = np.asarray(context, dtype=np.float32)
    W_in = np.asarray(W_in, dtype=np.float32)
    W_out = np.asarray(W_out, dtype=np.float32)
    ae = np.asarray(ae, dtype=np.float32).reshape(B)
    ab = np.asarray(ab, dtype=np.float32).reshape(B)

    # Replicated weights, host-transposed for natural SBUF tiling.
    wint = np.ascontiguousarray(W_in.T.reshape(NDC, P, D))
    import ml_dtypes
    woutt = np.ascontiguousarray(
        W_out.T.reshape(NCC, P, D2).astype(ml_dtypes.bfloat16)
    )

    # delta in [p, c] layout: delta[c*128+p] = L-1-(c*128+p)
    lidx = (np.arange(NLC)[None, :] * P + np.arange(P)[:, None]).astype(np.float32)
    delta_pc = (L - 1) - lidx                                    # [128, 16]

    in_maps = []
    for m in range(NCORES):
        bs = slice(m * BPC, (m + 1) * BPC)
        q = query[bs, 0, :]                                       # [4, 1024]
        qt = np.ascontiguousarray(
            q.T.reshape(NDC, P, BPC).transpose(1, 0, 2).reshape(P, NDC * BPC)
        )
        ctx = np.ascontiguousarray(context[bs].reshape(BPC, NLC, P, D))
        ae_m = ae[bs]
        ab_m = ab[bs]
        w2b = (ae_m[:, None, None]
               * np.exp(-ab_m[:, None, None] * delta_pc[None])).astype(np.float32)
        in_maps.append({
            "qt": qt,
            "ctx": ctx,
            "wint": wint,
            "woutt": woutt,
            "w2b": np.ascontiguousarray(w2b),
        })
    return in_maps


def _assemble(results):
    out = np.empty((B, 1, D2), dtype=np.float32)
    attn = np.empty((B, 1, L), dtype=np.float32)
    for m in range(NCORES):
        o = results[m]["out_o"]                       # [128, 16] = [p,(ot,b)]
        a = results[m]["attn_o"]                      # [4, 128, 16] = [b,p,c]
        out[m * BPC:(m + 1) * BPC, 0, :] = (
            o.reshape(P, NOT, BPC).transpose(2, 1, 0).reshape(BPC, D2)
        )
        attn[m * BPC:(m + 1) * BPC, 0, :] = (
            a.transpose(0, 2, 1).reshape(BPC, L)
        )
    return out, attn


def kernel(query, context, W_in, W_out, ae, ab):
    nc = _get_nc()
    in_maps = _prep_in_maps(query, context, W_in, W_out, ae, ab)
    res = run_bass_kernel_spmd(nc, in_maps, core_ids=list(range(NCORES)))
    return _assemble(res.results)


if __name__ == "__main__":
    # quick self-run against reference
    import reference

    inputs = {k: np.asarray(v) for k, v in reference.setup_inputs().items()}
    got_out, got_attn = kernel(**inputs)
    exp_out, exp_attn = reference.reference(**reference.setup_inputs())
    exp_out, exp_attn = np.asarray(exp_out), np.asarray(exp_attn)
    for name, g, e in (("out", got_out, exp_out), ("attn", got_attn, exp_attn)):
        rel = np.linalg.norm(g - e) / np.linalg.norm(e)
        print(f"{name}: rel_err={rel:.3e} max={np.abs(g - e).max():.3e}")


# revision 16
# speedup vs baseline: 1.3314x; 1.0249x over previous
"""Trainium2 Bass kernel for nn_Attention_65317862637882.

Data-parallel over batch (B=32) on 8 NeuronCores (4 batches/core).

Math (per batch, O=1):
  q      = W_in @ query                      [1024]
  scores = context @ q                       [2048]
  attn   = softmax(scores)
  bt     = exp(-ab * delta_t),  delta_t[l] = L-1-l
  mix    = attn * context^T                  [1024, 2048]
  term2  = relu(ae * mix * bt)
  mixed  = sum_l(term2 + mix)                [1024]
  out    = tanh(W_out @ [mixed; q])          [512]

Key reformulation: ae, attn, bt >= 0, so
  sum_l relu(ae*attn[l]*bt[l]*C[l,d]) = sum_l w2[l]*relu(C[l,d]),
  w2 = ae*attn*bt.  Hence
  mixed[d] = sum_l attn[l]*C[l,d] + sum_l w2[l]*relu(C[l,d])
i.e. two vector-stationary GEMVs contracting l (the SBUF partition dim of
naturally-laid-out context tiles) on the TensorEngine, plus one relu pass
(ScalarE).  scores needs the d-contraction, which the PE cannot do in this
layout; it runs on VectorE as a fused multiply+reduce (tensor_tensor_reduce)
against a broadcast copy of q.  Context is read from HBM exactly once.

Per-core engine budget (4 batches): DMA ~38MB (bound), DVE ~78us,
ACT ~68us, PE ~50us, GpSimd small.
"""

import os
import sys

import numpy as np

_HERE = os.path.dirname(os.path.abspath(__file__))
if _HERE not in sys.path:
    sys.path.insert(0, _HERE)

import concourse.bacc as bacc
import concourse.bass as bass  # noqa: F401
import concourse.mybir as mybir
import concourse.tile as tile
from concourse.bass_isa import ReduceOp  # noqa: E402
from concourse.bass_utils import run_bass_kernel_spmd

try:
    from tile_patch import legalize_single_wait
except ImportError:
    # kernel.py must be self-contained: inline fallback of the walrus
    # single-sync-wait legalization (see tile_patch.py).
    _uid = [0]

    def legalize_single_wait(nc):
        for fn in nc.m.functions:
            for bb in fn.blocks:
                new_insts = []
                changed = False
                for inst in bb.instructions:
                    si = inst.sync_info
                    if si is not None and len(si.on_wait) > 1:
                        waits = list(si.on_wait)
                        for w in waits[:-1]:
                            _uid[0] += 1
                            nop = mybir.InstNoOp(
                                name=f"I-swl-{_uid[0]}",
                                engine=inst.engine,
                                sync_info=mybir.SyncInfo(on_wait=[w], on_update=[]),
                            )
                            new_insts.append(nop)
                        si.on_wait = waits[-1:]
                        changed = True
                    new_insts.append(inst)
                if changed:
                    bb.instructions = new_insts


NCORES = 8
B, L, D, D2 = 32, 2048, 1024, 512
BPC = B // NCORES            # batches per core
P = 128                      # partitions
NLC = L // P                 # 16 l-chunks
NDC = D // P                 # 8 d-chunks
NCC = 2 * D // P             # 16 c-chunks of combined
NOT = D2 // P                # 4 o-tiles

F32 = mybir.dt.float32
BF16 = mybir.dt.bfloat16
AF = mybir.ActivationFunctionType
ALU = mybir.AluOpType
AX = mybir.AxisListType


def build_bass():
    nc = bacc.Bacc("TRN2", target_bir_lowering=False)

    qt_d = nc.declare_dram_parameter("qt", [P, NDC * BPC], F32, isOutput=False)          # [p,(dc,b)]
    ctx_d = nc.declare_dram_parameter("ctx", [BPC, NLC, P, D], F32, isOutput=False)
    wint_d = nc.declare_dram_parameter("wint", [NDC, P, D], F32, isOutput=False)         # W_in.T
    woutt_d = nc.declare_dram_parameter("woutt", [NCC, P, D2], BF16, isOutput=False)    # W_out.T bf16
    w2b_d = nc.declare_dram_parameter("w2b", [BPC, P, NLC], F32, isOutput=False)         # ae*exp(-ab*delta)
    out_d = nc.declare_dram_parameter("out_o", [P, NOT * BPC], F32, isOutput=True)
    attn_d = nc.declare_dram_parameter("attn_o", [BPC, P, NLC], F32, isOutput=True)

    with tile.TileContext(nc) as tc:
        with (
            tc.tile_pool(name="wts", bufs=3) as kpool,
            tc.tile_pool(name="singles", bufs=1) as qpool,
            tc.tile_pool(name="cdata", bufs=20) as cpool,
            tc.tile_pool(name="rdata", bufs=20) as rpool,
            tc.tile_pool(name="qb", bufs=2) as qbpool,
            tc.tile_pool(name="smalls", bufs=6) as spool,
            tc.tile_pool(name="psA", bufs=1, space="PSUM") as psa,
            tc.tile_pool(name="psB", bufs=1, space="PSUM") as psb,
        ):
            # ---- W_in GEMM, both output layouts --------------------------
            qt = qpool.tile([P, NDC * BPC], F32)
            nc.sync.dma_start(out=qt[:], in_=qt_d[:])

            qrow_ps = psa.tile([BPC, D], F32)      # q as rows (for broadcast)
            for dc in range(NDC):
                w = kpool.tile([P, D], F32)
                nc.sync.dma_start(out=w[:], in_=wint_d[dc])
                lhs_q = qt[:, dc * BPC:(dc + 1) * BPC]
                nc.tensor.matmul(qrow_ps[:, 0:512], lhs_q, w[:, 0:512],
                                 start=(dc == 0), stop=(dc == NDC - 1))
                nc.tensor.matmul(qrow_ps[:, 512:1024], lhs_q, w[:, 512:1024],
                                 start=(dc == 0), stop=(dc == NDC - 1))

            q_rows = qpool.tile([BPC, D], F32)
            nc.vector.tensor_copy(q_rows[:], qrow_ps[:])

            w2b_sb = qpool.tile([P, BPC * NLC], F32)
            for b in range(BPC):
                nc.scalar.dma_start(out=w2b_sb[:, b * NLC:(b + 1) * NLC],
                                    in_=w2b_d[b])

            ones1 = qpool.tile([P, 1], F32)
            nc.vector.memset(ones1[:], 1.0)
            ones11 = qpool.tile([1, 1], F32)
            nc.vector.memset(ones11[:], 1.0)
            warm_ps = psa.tile([1, P], F32)
            comb_ps = psa.tile([P, NCC, BPC], F32)

            # ---- per-batch main pipeline ---------------------------------
            for b in range(BPC):
                qrow_b = spool.tile([1, D], F32)
                nc.scalar.dma_start(out=qrow_b[:], in_=q_rows[b:b + 1, :])
                for j in range(NDC):
                    nc.tensor.matmul(
                        comb_ps[:, NDC + j, b:b + 1],
                        qrow_b[:, j * P:(j + 1) * P], ones11[:],
                        start=(b == 0 and j == 0), stop=True,
                    )
                qb = qbpool.tile([P, D], F32)
                nc.gpsimd.partition_broadcast(qb[:], qrow_b[:])

                scores = spool.tile([P, NLC], F32)
                c_tiles = []
                r_tiles = []
                for i in range(NLC):
                    ct = cpool.tile([P, D], F32)
                    nc.sync.dma_start(out=ct[:], in_=ctx_d[b, i])
                    rt = rpool.tile([P, D], BF16)
                    nc.scalar.activation(rt[:], ct[:], AF.Relu)
                    scr = qbpool.tile([P, D], F32, tag="amr_scr")
                    nc.vector.affine_mul_reduce(
                        scr[:], scores[:, i:i + 1], ct[:], qb[:], 1.0, 0.0,
                    )
                    # keep the PE HAM-warm: one cheap matmul per arriving chunk
                    nc.tensor.matmul(warm_ps[:, 0:8], ones1[:], ct[:, 0:8],
                                     start=True, stop=True)
                    c_tiles.append(ct)
                    r_tiles.append(rt)

                # softmax over all 2048 scores (layout [128, 16])
                mx = spool.tile([P, 1], F32)
                nc.vector.tensor_reduce(mx[:], scores[:], axis=AX.X, op=ALU.max)
                nc.gpsimd.partition_all_reduce(mx[:], mx[:], P, ReduceOp.max)
                negmx = spool.tile([P, 1], F32)
                nc.vector.tensor_scalar_mul(negmx[:], mx[:], -1.0)
                ex = spool.tile([P, NLC], F32)
                nc.scalar.activation(ex[:], scores[:], AF.Exp, bias=negmx[:])
                sm = spool.tile([P, 1], F32)
                nc.vector.tensor_reduce(sm[:], ex[:], axis=AX.X, op=ALU.add)
                nc.gpsimd.partition_all_reduce(sm[:], sm[:], P, ReduceOp.add)
                rinv = spool.tile([P, 1], F32)
                nc.vector.reciprocal(rinv[:], sm[:])
                attn = spool.tile([P, NLC], F32)
                nc.vector.tensor_scalar_mul(attn[:], ex[:], rinv[:])
                nc.scalar.dma_start(out=attn_d[b], in_=attn[:])

                w2 = spool.tile([P, NLC], F32)
                nc.vector.tensor_mul(w2[:], attn[:],
                                     w2b_sb[:, b * NLC:(b + 1) * NLC])
                w2bf = spool.tile([P, NLC], BF16)
                nc.vector.tensor_copy(w2bf[:], w2[:])

                # mixed[d] = sum_l attn[l]*C[l,d] + w2[l]*relu(C)[l,d]
                # moving-operand GEMV: vector stationary (1-col LDW), C/R
                # tiles stream through the PE; out rows [1, 1024] in PSUM.
                mixed_ps = psb.tile([1, D], F32)
                for h in range(2):          # two 512-col banks
                    sl = slice(h * 512, (h + 1) * 512)
                    for i in range(NLC):
                        nc.tensor.matmul(
                            mixed_ps[:, sl],
                            attn[:, i:i + 1],
                            c_tiles[i][:, sl],
                            start=(i == 0), stop=False,
                        )
                    for i in range(NLC):
                        nc.tensor.matmul(
                            mixed_ps[:, sl],
                            w2bf[:, i:i + 1],
                            r_tiles[i][:, sl],
                            start=False, stop=(i == NLC - 1),
                        )
                # scatter row -> [128 d-part, 8 chunks] via K=1 matmuls
                mrow = spool.tile([1, D], F32)
                nc.vector.tensor_copy(mrow[:], mixed_ps[:])
                for j in range(NDC):
                    nc.tensor.matmul(
                        comb_ps[:, j, b:b + 1],
                        mrow[:, j * P:(j + 1) * P], ones11[:],
                        start=False, stop=True,
                    )

            # ---- W_out GEMM (batched) + tanh -----------------------------
            comb_bf = qpool.tile([P, NCC, BPC], BF16)
            nc.vector.tensor_copy(comb_bf[:], comb_ps[:])
            ops_ps = psa.tile([P, NOT * BPC], F32)
            for cc in range(NCC):
                wo = kpool.tile([P, D2], BF16)
                nc.sync.dma_start(out=wo[:], in_=woutt_d[cc])
                for ot in range(NOT):
                    nc.tensor.matmul(
                        ops_ps[:, ot * BPC:(ot + 1) * BPC],
                        wo[:, ot * P:(ot + 1) * P],
                        comb_bf[:, cc, :],
                        start=(cc == 0 and ot == 0), stop=(cc == NCC - 1),
                    )
            out_sb = qpool.tile([P, NOT * BPC], F32)
            nc.scalar.activation(out_sb[:], ops_ps[:], AF.Tanh)
            nc.scalar.dma_start(out=out_d[:], in_=out_sb[:])

    nc.compile()
    legalize_single_wait(nc)
    return nc


_NC_CACHE = None


def _get_nc():
    global _NC_CACHE
    if _NC_CACHE is None:
        _NC_CACHE = build_bass()
    return _NC_CACHE


def _prep_in_maps(query, context, W_in, W_out, ae, ab):
    query = np.asarray(query, dtype=np.float32)
    context = np.asarray(context, dtype=np.float32)
    W_in = np.asarray(W_in, dtype=np.float32)
    W_out = np.asarray(W_out, dtype=np.float32)
    ae = np.asarray(ae, dtype=np.float32).reshape(B)
    ab = np.asarray(ab, dtype=np.float32).reshape(B)

    # Replicated weights, host-transposed for natural SBUF tiling.
    wint = np.ascontiguousarray(W_in.T.reshape(NDC, P, D))
    import ml_dtypes
    woutt = np.ascontiguousarray(
        W_out.T.reshape(NCC, P, D2).astype(ml_dtypes.bfloat16)
    )

    # delta in [p, c] layout: delta[c*128+p] = L-1-(c*128+p)
    lidx = (np.arange(NLC)[None, :] * P + np.arange(P)[:, None]).astype(np.float32)
    delta_pc = (L - 1) - lidx                                    # [128, 16]

    in_maps = []
    for m in range(NCORES):
        bs = slice(m * BPC, (m + 1) * BPC)
        q = query[bs, 0, :]                                       # [4, 1024]
        qt = np.ascontiguousarray(
            q.T.reshape(NDC, P, BPC).transpose(1, 0, 2).reshape(P, NDC * BPC)
        )
        ctx = np.ascontiguousarray(context[bs].reshape(BPC, NLC, P, D))
        ae_m = ae[bs]
        ab_m = ab[bs]
        w2b = (ae_m[:, None, None]
               * np.exp(-ab_m[:, None, None] * delta_pc[None])).astype(np.float32)
        in_maps.append({
            "qt": qt,
            "ctx": ctx,
            "wint": wint,
            "woutt": woutt,
            "w2b": np.ascontiguousarray(w2b),
        })
    return in_maps


def _assemble(results):
    out = np.empty((B, 1, D2), dtype=np.float32)
    attn = np.empty((B, 1, L), dtype=np.float32)
    for m in range(NCORES):
        o = results[m]["out_o"]                       # [128, 16] = [p,(ot,b)]
        a = results[m]["attn_o"]                      # [4, 128, 16] = [b,p,c]
        out[m * BPC:(m + 1) * BPC, 0, :] = (
            o.reshape(P, NOT, BPC).transpose(2, 1, 0).reshape(BPC, D2)
        )
        attn[m * BPC:(m + 1) * BPC, 0, :] = (
            a.transpose(0, 2, 1).reshape(BPC, L)
        )
    return out, attn


def kernel(query, context, W_in, W_out, ae, ab):
    nc = _get_nc()
    in_maps = _prep_in_maps(query, context, W_in, W_out, ae, ab)
    res = run_bass_kernel_spmd(nc, in_maps, core_ids=list(range(NCORES)))
    return _assemble(res.results)


if __name__ == "__main__":
    # quick self-run against reference
    import reference

    inputs = {k: np.asarray(v) for k, v in reference.setup_inputs().items()}
    got_out, got_attn = kernel(**inputs)
    exp_out, exp_attn = reference.reference(**reference.setup_inputs())
    exp_out, exp_attn = np.asarray(exp_out), np.asarray(exp_attn)
    for name, g, e in (("out", got_out, exp_out), ("attn", got_attn, exp_attn)):
        rel = np.linalg.norm(g - e) / np.linalg.norm(e)
        print(f"{name}: rel_err={rel:.3e} max={np.abs(g - e).max():.3e}")


# revision 18
# speedup vs baseline: 1.4549x; 1.0928x over previous
"""Trainium2 Bass kernel for nn_Attention_65317862637882.

Data-parallel over batch (B=32) on 8 NeuronCores (4 batches/core).

Math (per batch, O=1):
  q      = W_in @ query                      [1024]
  scores = context @ q                       [2048]
  attn   = softmax(scores)
  bt     = exp(-ab * delta_t),  delta_t[l] = L-1-l
  mix    = attn * context^T                  [1024, 2048]
  term2  = relu(ae * mix * bt)
  mixed  = sum_l(term2 + mix)                [1024]
  out    = tanh(W_out @ [mixed; q])          [512]

Key reformulation: ae, attn, bt >= 0, so
  sum_l relu(ae*attn[l]*bt[l]*C[l,d]) = sum_l w2[l]*relu(C[l,d]),
  w2 = ae*attn*bt.  Hence
  mixed[d] = sum_l attn[l]*C[l,d] + sum_l w2[l]*relu(C[l,d])
i.e. two vector-stationary GEMVs contracting l (the SBUF partition dim of
naturally-laid-out context tiles) on the TensorEngine, plus one relu pass
(ScalarE).  scores needs the d-contraction, which the PE cannot do in this
layout; it runs on VectorE as a fused multiply+reduce (tensor_tensor_reduce)
against a broadcast copy of q.  Context is read from HBM exactly once.

Per-core engine budget (4 batches): DMA ~38MB (bound), DVE ~78us,
ACT ~68us, PE ~50us, GpSimd small.
"""

import os
import sys

import numpy as np

_HERE = os.path.dirname(os.path.abspath(__file__))
if _HERE not in sys.path:
    sys.path.insert(0, _HERE)

import concourse.bacc as bacc
import concourse.bass as bass  # noqa: F401
import concourse.mybir as mybir
import concourse.tile as tile
from concourse.bass_isa import ReduceOp  # noqa: E402
from concourse.bass_utils import run_bass_kernel_spmd

try:
    from tile_patch import legalize_single_wait
except ImportError:
    # kernel.py must be self-contained: inline fallback of the walrus
    # single-sync-wait legalization (see tile_patch.py).
    _uid = [0]

    def legalize_single_wait(nc):
        for fn in nc.m.functions:
            for bb in fn.blocks:
                new_insts = []
                changed = False
                for inst in bb.instructions:
                    si = inst.sync_info
                    if si is not None and len(si.on_wait) > 1:
                        waits = list(si.on_wait)
                        for w in waits[:-1]:
                            _uid[0] += 1
                            nop = mybir.InstNoOp(
                                name=f"I-swl-{_uid[0]}",
                                engine=inst.engine,
                                sync_info=mybir.SyncInfo(on_wait=[w], on_update=[]),
                            )
                            new_insts.append(nop)
                        si.on_wait = waits[-1:]
                        changed = True
                    new_insts.append(inst)
                if changed:
                    bb.instructions = new_insts


NCORES = 8
B, L, D, D2 = 32, 2048, 1024, 512
BPC = B // NCORES            # batches per core
P = 128                      # partitions
NLC = L // P                 # 16 l-chunks
NDC = D // P                 # 8 d-chunks
NCC = 2 * D // P             # 16 c-chunks of combined
NOT = D2 // P                # 4 o-tiles

F32 = mybir.dt.float32
BF16 = mybir.dt.bfloat16
AF = mybir.ActivationFunctionType
ALU = mybir.AluOpType
AX = mybir.AxisListType


def build_bass():
    nc = bacc.Bacc("TRN2", target_bir_lowering=False)

    qt_d = nc.declare_dram_parameter("qt", [P, NDC * BPC], F32, isOutput=False)          # [p,(dc,b)]
    ctx_d = nc.declare_dram_parameter("ctx", [BPC, NLC, P, D], F32, isOutput=False)
    wint_d = nc.declare_dram_parameter("wint", [NDC, P, D], F32, isOutput=False)         # W_in.T
    woutt_d = nc.declare_dram_parameter("woutt", [NCC, P, D2], BF16, isOutput=False)    # W_out.T bf16
    w2b_d = nc.declare_dram_parameter("w2b", [BPC, P, NLC], F32, isOutput=False)         # ae*exp(-ab*delta)
    out_d = nc.declare_dram_parameter("out_o", [P, NOT * BPC], F32, isOutput=True)
    attn_d = nc.declare_dram_parameter("attn_o", [BPC, P, NLC], F32, isOutput=True)

    with tile.TileContext(nc) as tc:
        with (
            tc.tile_pool(name="wts", bufs=3) as kpool,
            tc.tile_pool(name="singles", bufs=1) as qpool,
            tc.tile_pool(name="cdata", bufs=22) as cpool,
            tc.tile_pool(name="rdata", bufs=24) as rpool,
            tc.tile_pool(name="qb", bufs=2) as qbpool,
            tc.tile_pool(name="rows", bufs=2) as rowpool,
            tc.tile_pool(name="smalls", bufs=6) as spool,
            tc.tile_pool(name="psA", bufs=1, space="PSUM") as psa,
            tc.tile_pool(name="psB", bufs=2, space="PSUM") as psb,
        ):
            # ---- W_in GEMM, both output layouts --------------------------
            qt = qpool.tile([P, NDC * BPC], F32)
            nc.sync.dma_start(out=qt[:], in_=qt_d[:])

            qrow_ps = psa.tile([BPC, D], F32)      # q as rows (for broadcast)
            for dc in range(NDC):
                w = kpool.tile([P, D], F32)
                nc.sync.dma_start(out=w[:], in_=wint_d[dc])
                lhs_q = qt[:, dc * BPC:(dc + 1) * BPC]
                nc.tensor.matmul(qrow_ps[:, 0:512], lhs_q, w[:, 0:512],
                                 start=(dc == 0), stop=(dc == NDC - 1))
                nc.tensor.matmul(qrow_ps[:, 512:1024], lhs_q, w[:, 512:1024],
                                 start=(dc == 0), stop=(dc == NDC - 1))

            q_rows = qpool.tile([BPC, D], F32)
            nc.vector.tensor_copy(q_rows[:], qrow_ps[:])

            w2b_sb = qpool.tile([P, BPC * NLC], F32)
            for b in range(BPC):
                nc.scalar.dma_start(out=w2b_sb[:, b * NLC:(b + 1) * NLC],
                                    in_=w2b_d[b])

            ones1 = qpool.tile([P, 1], F32)
            nc.vector.memset(ones1[:], 1.0)
            ones11 = qpool.tile([1, 1], F32)
            nc.vector.memset(ones11[:], 1.0)
            comb_ps = psa.tile([P, NCC, BPC], F32)
            ops_ps = psa.tile([P, NOT * BPC], F32)

            # ---- per-batch main pipeline ---------------------------------
            for b in range(BPC):
                qrow_b = rowpool.tile([1, D], F32)
                nc.scalar.dma_start(out=qrow_b[:], in_=q_rows[b:b + 1, :])
                for j in range(NDC):
                    nc.tensor.matmul(
                        comb_ps[:, NDC + j, b:b + 1],
                        qrow_b[:, j * P:(j + 1) * P], ones11[:],
                        start=(b == 0 and j == 0), stop=True,
                    )
                qb = qbpool.tile([P, D], F32)
                nc.gpsimd.partition_broadcast(qb[:], qrow_b[:])

                scores = spool.tile([P, NLC], F32)
                c_tiles = []
                r_tiles = []
                for i in range(NLC):
                    ct = cpool.tile([P, D], F32)
                    nc.sync.dma_start(out=ct[:], in_=ctx_d[b, i])
                    rt = rpool.tile([P, D], BF16)
                    nc.scalar.activation(rt[:], ct[:], AF.Relu)
                    scr = qbpool.tile([P, D], F32, tag="amr_scr")
                    nc.vector.affine_mul_reduce(
                        scr[:], scores[:, i:i + 1], ct[:], qb[:], 1.0, 0.0,
                    )
                    # keep the PE HAM-warm: one cheap matmul per arriving chunk
                    nc.tensor.matmul(ops_ps[0:1, 0:8], ones1[:], ct[:, 0:8],
                                     start=True, stop=True)
                    c_tiles.append(ct)
                    r_tiles.append(rt)

                # softmax over all 2048 scores (layout [128, 16])
                mx = spool.tile([P, 1], F32)
                nc.vector.tensor_reduce(mx[:], scores[:], axis=AX.X, op=ALU.max)
                nc.gpsimd.partition_all_reduce(mx[:], mx[:], P, ReduceOp.max)
                negmx = spool.tile([P, 1], F32)
                nc.vector.tensor_scalar_mul(negmx[:], mx[:], -1.0)
                ex = spool.tile([P, NLC], F32)
                nc.scalar.activation(ex[:], scores[:], AF.Exp, bias=negmx[:])
                sm = spool.tile([P, 1], F32)
                nc.vector.tensor_reduce(sm[:], ex[:], axis=AX.X, op=ALU.add)
                nc.gpsimd.partition_all_reduce(sm[:], sm[:], P, ReduceOp.add)
                rinv = spool.tile([P, 1], F32)
                nc.vector.reciprocal(rinv[:], sm[:])
                attn = spool.tile([P, NLC], F32)
                nc.vector.tensor_scalar_mul(attn[:], ex[:], rinv[:])
                nc.scalar.dma_start(out=attn_d[b], in_=attn[:])

                w2 = spool.tile([P, NLC], F32)
                nc.vector.tensor_mul(w2[:], attn[:],
                                     w2b_sb[:, b * NLC:(b + 1) * NLC])
                w2bf = spool.tile([P, NLC], BF16)
                nc.vector.tensor_copy(w2bf[:], w2[:])

                # mixed[d] = sum_l attn[l]*C[l,d] + w2[l]*relu(C)[l,d]
                # moving-operand GEMV: vector stationary (1-col LDW), C/R
                # tiles stream through the PE; out rows [1, 1024] in PSUM.
                mixed_ps = psb.tile([1, D], F32)
                for i in range(NLC):
                    for h in range(2):      # two 512-col banks
                        sl = slice(h * 512, (h + 1) * 512)
                        nc.tensor.matmul(
                            mixed_ps[:, sl],
                            attn[:, i:i + 1],
                            c_tiles[i][:, sl],
                            start=(i == 0), stop=False,
                        )
                        nc.tensor.matmul(
                            mixed_ps[:, sl],
                            w2bf[:, i:i + 1],
                            r_tiles[i][:, sl],
                            start=False, stop=(i == NLC - 1),
                        )
                # scatter row -> [128 d-part, 8 chunks] via K=1 matmuls
                mrow = rowpool.tile([1, D], F32)
                nc.vector.tensor_copy(mrow[:], mixed_ps[:])
                for j in range(NDC):
                    nc.tensor.matmul(
                        comb_ps[:, j, b:b + 1],
                        mrow[:, j * P:(j + 1) * P], ones11[:],
                        start=False, stop=True,
                    )

            # ---- W_out GEMM (batched) + tanh -----------------------------
            comb_bf = qpool.tile([P, NCC, BPC], BF16)
            nc.vector.tensor_copy(comb_bf[:], comb_ps[:])
            for cc in range(NCC):
                wo = kpool.tile([P, D2], BF16)
                nc.sync.dma_start(out=wo[:], in_=woutt_d[cc])
                for ot in range(NOT):
                    nc.tensor.matmul(
                        ops_ps[:, ot * BPC:(ot + 1) * BPC],
                        wo[:, ot * P:(ot + 1) * P],
                        comb_bf[:, cc, :],
                        start=(cc == 0 and ot == 0), stop=(cc == NCC - 1),
                    )
            out_sb = qpool.tile([P, NOT * BPC], F32)
            nc.scalar.activation(out_sb[:], ops_ps[:], AF.Tanh)
            nc.scalar.dma_start(out=out_d[:], in_=out_sb[:])

    nc.compile()
    legalize_single_wait(nc)
    return nc


_NC_CACHE = None


def _get_nc():
    global _NC_CACHE
    if _NC_CACHE is None:
        _NC_CACHE = build_bass()
    return _NC_CACHE


def _prep_in_maps(query, context, W_in, W_out, ae, ab):
    query = np.asarray(query, dtype=np.float32)
    context = np.asarray(context, dtype=np.float32)
    W_in = np.asarray(W_in, dtype=np.float32)
    W_out = np.asarray(W_out, dtype=np.float32)
    ae = np.asarray(ae, dtype=np.float32).reshape(B)
    ab = np.asarray(ab, dtype=np.float32).reshape(B)

    # Replicated weights, host-transposed for natural SBUF tiling.
    wint = np.ascontiguousarray(W_in.T.reshape(NDC, P, D))
    import ml_dtypes
    woutt = np.ascontiguousarray(
        W_out.T.reshape(NCC, P, D2).astype(ml_dtypes.bfloat16)
    )

    # delta in [p, c] layout: delta[c*128+p] = L-1-(c*128+p)
    lidx = (np.arange(NLC)[None, :] * P + np.arange(P)[:, None]).astype(np.float32)
    delta_pc = (L - 1) - lidx                                    # [128, 16]

    in_maps = []
    for m in range(NCORES):
        bs = slice(m * BPC, (m + 1) * BPC)
        q = query[bs, 0, :]                                       # [4, 1024]
        qt = np.ascontiguousarray(
            q.T.reshape(NDC, P, BPC).transpose(1, 0, 2).reshape(P, NDC * BPC)
        )
        ctx = np.ascontiguousarray(context[bs].reshape(BPC, NLC, P, D))
        ae_m = ae[bs]
        ab_m = ab[bs]
        w2b = (ae_m[:, None, None]
               * np.exp(-ab_m[:, None, None] * delta_pc[None])).astype(np.float32)
        in_maps.append({
            "qt": qt,
            "ctx": ctx,
            "wint": wint,
            "woutt": woutt,
            "w2b": np.ascontiguousarray(w2b),
        })
    return in_maps


def _assemble(results):
    out = np.empty((B, 1, D2), dtype=np.float32)
    attn = np.empty((B, 1, L), dtype=np.float32)
    for m in range(NCORES):
        o = results[m]["out_o"]                       # [128, 16] = [p,(ot,b)]
        a = results[m]["attn_o"]                      # [4, 128, 16] = [b,p,c]
        out[m * BPC:(m + 1) * BPC, 0, :] = (
            o.reshape(P, NOT, BPC).transpose(2, 1, 0).reshape(BPC, D2)
        )
        attn[m * BPC:(m + 1) * BPC, 0, :] = (
            a.transpose(0, 2, 1).reshape(BPC, L)
        )
    return out, attn


def kernel(query, context, W_in, W_out, ae, ab):
    nc = _get_nc()
    in_maps = _prep_in_maps(query, context, W_in, W_out, ae, ab)
    res = run_bass_kernel_spmd(nc, in_maps, core_ids=list(range(NCORES)))
    return _assemble(res.results)


if __name__ == "__main__":
    # quick self-run against reference
    import reference

    inputs = {k: np.asarray(v) for k, v in reference.setup_inputs().items()}
    got_out, got_attn = kernel(**inputs)
    exp_out, exp_attn = reference.reference(**reference.setup_inputs())
    exp_out, exp_attn = np.asarray(exp_out), np.asarray(exp_attn)
    for name, g, e in (("out", got_out, exp_out), ("attn", got_attn, exp_attn)):
        rel = np.linalg.norm(g - e) / np.linalg.norm(e)
        print(f"{name}: rel_err={rel:.3e} max={np.abs(g - e).max():.3e}")


# revision 19
# speedup vs baseline: 1.7710x; 1.2172x over previous
"""Trainium2 Bass kernel for nn_Attention_65317862637882.

Data-parallel over batch (B=32) on 8 NeuronCores (4 batches/core).

Math (per batch, O=1):
  q      = W_in @ query                      [1024]
  scores = context @ q                       [2048]
  attn   = softmax(scores)
  bt     = exp(-ab * delta_t),  delta_t[l] = L-1-l
  mix    = attn * context^T                  [1024, 2048]
  term2  = relu(ae * mix * bt)
  mixed  = sum_l(term2 + mix)                [1024]
  out    = tanh(W_out @ [mixed; q])          [512]

Key reformulation: ae, attn, bt >= 0, so
  sum_l relu(ae*attn[l]*bt[l]*C[l,d]) = sum_l w2[l]*relu(C[l,d]),
  w2 = ae*attn*bt.  Hence
  mixed[d] = sum_l attn[l]*C[l,d] + sum_l w2[l]*relu(C[l,d])
i.e. two vector-stationary GEMVs contracting l (the SBUF partition dim of
naturally-laid-out context tiles) on the TensorEngine, plus one relu pass
(ScalarE).  scores needs the d-contraction, which the PE cannot do in this
layout; it runs on VectorE as a fused multiply+reduce (tensor_tensor_reduce)
against a broadcast copy of q.  Context is read from HBM exactly once.

Per-core engine budget (4 batches): DMA ~38MB (bound), DVE ~78us,
ACT ~68us, PE ~50us, GpSimd small.
"""

import os
import sys

import numpy as np

_HERE = os.path.dirname(os.path.abspath(__file__))
if _HERE not in sys.path:
    sys.path.insert(0, _HERE)

import concourse.bacc as bacc
import concourse.bass as bass  # noqa: F401
import concourse.mybir as mybir
import concourse.tile as tile
from concourse.bass_isa import ReduceOp  # noqa: E402
from concourse.bass_utils import run_bass_kernel_spmd

try:
    from tile_patch import legalize_single_wait
except ImportError:
    # kernel.py must be self-contained: inline fallback of the walrus
    # single-sync-wait legalization (see tile_patch.py).
    _uid = [0]

    def legalize_single_wait(nc):
        for fn in nc.m.functions:
            for bb in fn.blocks:
                new_insts = []
                changed = False
                for inst in bb.instructions:
                    si = inst.sync_info
                    if si is not None and len(si.on_wait) > 1:
                        waits = list(si.on_wait)
                        for w in waits[:-1]:
                            _uid[0] += 1
                            nop = mybir.InstNoOp(
                                name=f"I-swl-{_uid[0]}",
                                engine=inst.engine,
                                sync_info=mybir.SyncInfo(on_wait=[w], on_update=[]),
                            )
                            new_insts.append(nop)
                        si.on_wait = waits[-1:]
                        changed = True
                    new_insts.append(inst)
                if changed:
                    bb.instructions = new_insts


NCORES = 8
B, L, D, D2 = 32, 2048, 1024, 512
BPC = B // NCORES            # batches per core
P = 128                      # partitions
NLC = L // P                 # 16 l-chunks
NDC = D // P                 # 8 d-chunks
NCC = 2 * D // P             # 16 c-chunks of combined
NOT = D2 // P                # 4 o-tiles

F32 = mybir.dt.float32
BF16 = mybir.dt.bfloat16
AF = mybir.ActivationFunctionType
ALU = mybir.AluOpType
AX = mybir.AxisListType


def build_bass():
    nc = bacc.Bacc("TRN2", target_bir_lowering=False)

    qt_d = nc.declare_dram_parameter("qt", [P, NDC * BPC], F32, isOutput=False)          # [p,(dc,b)]
    ctx_d = nc.declare_dram_parameter("ctx", [BPC, NLC, P, D], F32, isOutput=False)
    wint_d = nc.declare_dram_parameter("wint", [NDC, P, D], F32, isOutput=False)         # W_in.T
    woutt_d = nc.declare_dram_parameter("woutt", [NCC, P, D2], BF16, isOutput=False)    # W_out.T bf16
    w2b_d = nc.declare_dram_parameter("w2b", [BPC, P, NLC], F32, isOutput=False)         # ae*exp(-ab*delta)
    out_d = nc.declare_dram_parameter("out_o", [P, NOT * BPC], F32, isOutput=True)
    attn_d = nc.declare_dram_parameter("attn_o", [BPC, P, NLC], F32, isOutput=True)

    with tile.TileContext(nc) as tc:
        with (
            tc.tile_pool(name="wts", bufs=3) as kpool,
            tc.tile_pool(name="singles", bufs=1) as qpool,
            tc.tile_pool(name="cdata", bufs=8) as cpool,
            tc.tile_pool(name="rdata", bufs=24) as rpool,
            tc.tile_pool(name="cbdata", bufs=24) as cbpool,
            tc.tile_pool(name="qb", bufs=2) as qbpool,
            tc.tile_pool(name="rows", bufs=2) as rowpool,
            tc.tile_pool(name="smalls", bufs=6) as spool,
            tc.tile_pool(name="psA", bufs=1, space="PSUM") as psa,
            tc.tile_pool(name="psB", bufs=2, space="PSUM") as psb,
        ):
            # ---- W_in GEMM, both output layouts --------------------------
            qt = qpool.tile([P, NDC * BPC], F32)
            nc.sync.dma_start(out=qt[:], in_=qt_d[:])

            qrow_ps = psa.tile([BPC, D], F32)      # q as rows (for broadcast)
            for dc in range(NDC):
                w = kpool.tile([P, D], F32)
                nc.sync.dma_start(out=w[:], in_=wint_d[dc])
                lhs_q = qt[:, dc * BPC:(dc + 1) * BPC]
                nc.tensor.matmul(qrow_ps[:, 0:512], lhs_q, w[:, 0:512],
                                 start=(dc == 0), stop=(dc == NDC - 1))
                nc.tensor.matmul(qrow_ps[:, 512:1024], lhs_q, w[:, 512:1024],
                                 start=(dc == 0), stop=(dc == NDC - 1))

            q_rows = qpool.tile([BPC, D], F32)
            nc.vector.tensor_copy(q_rows[:], qrow_ps[:])

            w2b_sb = qpool.tile([P, BPC * NLC], F32)
            for b in range(BPC):
                nc.scalar.dma_start(out=w2b_sb[:, b * NLC:(b + 1) * NLC],
                                    in_=w2b_d[b])

            ones1 = qpool.tile([P, 1], F32)
            nc.vector.memset(ones1[:], 1.0)
            ones11 = qpool.tile([1, 1], F32)
            nc.vector.memset(ones11[:], 1.0)
            comb_ps = psa.tile([P, NCC, BPC], F32)
            ops_ps = psa.tile([P, NOT * BPC], F32)

            # ---- per-batch main pipeline ---------------------------------
            for b in range(BPC):
                qrow_b = rowpool.tile([1, D], F32)
                nc.scalar.dma_start(out=qrow_b[:], in_=q_rows[b:b + 1, :])
                for j in range(NDC):
                    nc.tensor.matmul(
                        comb_ps[:, NDC + j, b:b + 1],
                        qrow_b[:, j * P:(j + 1) * P], ones11[:],
                        start=(b == 0 and j == 0), stop=True,
                    )
                qb = qbpool.tile([P, D], F32)
                nc.gpsimd.partition_broadcast(qb[:], qrow_b[:])

                scores = spool.tile([P, NLC], F32)
                cb_tiles = []
                r_tiles = []
                for i in range(NLC):
                    ct = cpool.tile([P, D], F32)
                    nc.sync.dma_start(out=ct[:], in_=ctx_d[b, i])
                    rt = rpool.tile([P, D], BF16)
                    nc.scalar.activation(rt[:], ct[:], AF.Relu)
                    scr = qbpool.tile([P, D], F32, tag="amr_scr")
                    nc.vector.affine_mul_reduce(
                        scr[:], scores[:, i:i + 1], ct[:], qb[:], 1.0, 0.0,
                    )
                    cbt = cbpool.tile([P, D], BF16)
                    nc.vector.tensor_copy(cbt[:], ct[:])
                    # keep the PE HAM-warm: one cheap matmul per arriving chunk
                    nc.tensor.matmul(ops_ps[0:1, 0:8], ones1[:], ct[:, 0:8],
                                     start=True, stop=True)
                    cb_tiles.append(cbt)
                    r_tiles.append(rt)

                # softmax over all 2048 scores (layout [128, 16])
                mx = spool.tile([P, 1], F32)
                nc.vector.tensor_reduce(mx[:], scores[:], axis=AX.X, op=ALU.max)
                nc.gpsimd.partition_all_reduce(mx[:], mx[:], P, ReduceOp.max)
                negmx = spool.tile([P, 1], F32)
                nc.vector.tensor_scalar_mul(negmx[:], mx[:], -1.0)
                ex = spool.tile([P, NLC], F32)
                nc.scalar.activation(ex[:], scores[:], AF.Exp, bias=negmx[:])
                sm = spool.tile([P, 1], F32)
                nc.vector.tensor_reduce(sm[:], ex[:], axis=AX.X, op=ALU.add)
                nc.gpsimd.partition_all_reduce(sm[:], sm[:], P, ReduceOp.add)
                rinv = spool.tile([P, 1], F32)
                nc.vector.reciprocal(rinv[:], sm[:])
                attn = spool.tile([P, NLC], F32)
                nc.vector.tensor_scalar_mul(attn[:], ex[:], rinv[:])
                nc.scalar.dma_start(out=attn_d[b], in_=attn[:])

                w2 = spool.tile([P, NLC], F32)
                nc.vector.tensor_mul(w2[:], attn[:],
                                     w2b_sb[:, b * NLC:(b + 1) * NLC])
                w2bf = spool.tile([P, NLC], BF16)
                nc.vector.tensor_copy(w2bf[:], w2[:])
                attn_bf = spool.tile([P, NLC], BF16)
                nc.vector.tensor_copy(attn_bf[:], attn[:])

                # mixed[d] = sum_l attn[l]*C[l,d] + w2[l]*relu(C)[l,d]
                # moving-operand GEMV: vector stationary (1-col LDW), C/R
                # tiles stream through the PE; out rows [1, 1024] in PSUM.
                mixed_ps = psb.tile([1, D], F32)
                for i in range(NLC):
                    for h in range(2):      # two 512-col banks
                        sl = slice(h * 512, (h + 1) * 512)
                        nc.tensor.matmul(
                            mixed_ps[:, sl],
                            attn_bf[:, i:i + 1],
                            cb_tiles[i][:, sl],
                            start=(i == 0), stop=False,
                        )
                        nc.tensor.matmul(
                            mixed_ps[:, sl],
                            w2bf[:, i:i + 1],
                            r_tiles[i][:, sl],
                            start=False, stop=(i == NLC - 1),
                        )
                # scatter row -> [128 d-part, 8 chunks] via K=1 matmuls
                mrow = rowpool.tile([1, D], F32)
                nc.vector.tensor_copy(mrow[:], mixed_ps[:])
                for j in range(NDC):
                    nc.tensor.matmul(
                        comb_ps[:, j, b:b + 1],
                        mrow[:, j * P:(j + 1) * P], ones11[:],
                        start=False, stop=True,
                    )

            # ---- W_out GEMM (batched) + tanh -----------------------------
            comb_bf = qpool.tile([P, NCC, BPC], BF16)
            nc.vector.tensor_copy(comb_bf[:], comb_ps[:])
            for cc in range(NCC):
                wo = kpool.tile([P, D2], BF16)
                nc.sync.dma_start(out=wo[:], in_=woutt_d[cc])
                for ot in range(NOT):
                    nc.tensor.matmul(
                        ops_ps[:, ot * BPC:(ot + 1) * BPC],
                        wo[:, ot * P:(ot + 1) * P],
                        comb_bf[:, cc, :],
                        start=(cc == 0 and ot == 0), stop=(cc == NCC - 1),
                    )
            out_sb = qpool.tile([P, NOT * BPC], F32)
            nc.scalar.activation(out_sb[:], ops_ps[:], AF.Tanh)
            nc.scalar.dma_start(out=out_d[:], in_=out_sb[:])

    nc.compile()
    legalize_single_wait(nc)
    return nc


_NC_CACHE = None


def _get_nc():
    global _NC_CACHE
    if _NC_CACHE is None:
        _NC_CACHE = build_bass()
    return _NC_CACHE


def _prep_in_maps(query, context, W_in, W_out, ae, ab):
    query = np.asarray(query, dtype=np.float32)
    context = np.asarray(context, dtype=np.float32)
    W_in = np.asarray(W_in, dtype=np.float32)
    W_out = np.asarray(W_out, dtype=np.float32)
    ae = np.asarray(ae, dtype=np.float32).reshape(B)
    ab = np.asarray(ab, dtype=np.float32).reshape(B)

    # Replicated weights, host-transposed for natural SBUF tiling.
    wint = np.ascontiguousarray(W_in.T.reshape(NDC, P, D))
    import ml_dtypes
    woutt = np.ascontiguousarray(
        W_out.T.reshape(NCC, P, D2).astype(ml_dtypes.bfloat16)
    )

    # delta in [p, c] layout: delta[c*128+p] = L-1-(c*128+p)
    lidx = (np.arange(NLC)[None, :] * P + np.arange(P)[:, None]).astype(np.float32)
    delta_pc = (L - 1) - lidx                                    # [128, 16]

    in_maps = []
    for m in range(NCORES):
        bs = slice(m * BPC, (m + 1) * BPC)
        q = query[bs, 0, :]                                       # [4, 1024]
        qt = np.ascontiguousarray(
            q.T.reshape(NDC, P, BPC).transpose(1, 0, 2).reshape(P, NDC * BPC)
        )
        ctx = np.ascontiguousarray(context[bs].reshape(BPC, NLC, P, D))
        ae_m = ae[bs]
        ab_m = ab[bs]
        w2b = (ae_m[:, None, None]
               * np.exp(-ab_m[:, None, None] * delta_pc[None])).astype(np.float32)
        in_maps.append({
            "qt": qt,
            "ctx": ctx,
            "wint": wint,
            "woutt": woutt,
            "w2b": np.ascontiguousarray(w2b),
        })
    return in_maps


def _assemble(results):
    out = np.empty((B, 1, D2), dtype=np.float32)
    attn = np.empty((B, 1, L), dtype=np.float32)
    for m in range(NCORES):
        o = results[m]["out_o"]                       # [128, 16] = [p,(ot,b)]
        a = results[m]["attn_o"]                      # [4, 128, 16] = [b,p,c]
        out[m * BPC:(m + 1) * BPC, 0, :] = (
            o.reshape(P, NOT, BPC).transpose(2, 1, 0).reshape(BPC, D2)
        )
        attn[m * BPC:(m + 1) * BPC, 0, :] = (
            a.transpose(0, 2, 1).reshape(BPC, L)
        )
    return out, attn


def kernel(query, context, W_in, W_out, ae, ab):
    nc = _get_nc()
    in_maps = _prep_in_maps(query, context, W_in, W_out, ae, ab)
    res = run_bass_kernel_spmd(nc, in_maps, core_ids=list(range(NCORES)))
    return _assemble(res.results)


if __name__ == "__main__":
    # quick self-run against reference
    import reference

    inputs = {k: np.asarray(v) for k, v in reference.setup_inputs().items()}
    got_out, got_attn = kernel(**inputs)
    exp_out, exp_attn = reference.reference(**reference.setup_inputs())
    exp_out, exp_attn = np.asarray(exp_out), np.asarray(exp_attn)
    for name, g, e in (("out", got_out, exp_out), ("attn", got_attn, exp_attn)):
        rel = np.linalg.norm(g - e) / np.linalg.norm(e)
        print(f"{name}: rel_err={rel:.3e} max={np.abs(g - e).max():.3e}")


# revision 20
# speedup vs baseline: 1.9333x; 1.0916x over previous
"""Trainium2 Bass kernel for nn_Attention_65317862637882.

Data-parallel over batch (B=32) on 8 NeuronCores (4 batches/core).

Math (per batch, O=1):
  q      = W_in @ query                      [1024]
  scores = context @ q                       [2048]
  attn   = softmax(scores)
  bt     = exp(-ab * delta_t),  delta_t[l] = L-1-l
  mix    = attn * context^T                  [1024, 2048]
  term2  = relu(ae * mix * bt)
  mixed  = sum_l(term2 + mix)                [1024]
  out    = tanh(W_out @ [mixed; q])          [512]

Key reformulation: ae, attn, bt >= 0, so
  sum_l relu(ae*attn[l]*bt[l]*C[l,d]) = sum_l w2[l]*relu(C[l,d]),
  w2 = ae*attn*bt.  Hence
  mixed[d] = sum_l attn[l]*C[l,d] + sum_l w2[l]*relu(C[l,d])
i.e. two vector-stationary GEMVs contracting l (the SBUF partition dim of
naturally-laid-out context tiles) on the TensorEngine, plus one relu pass
(ScalarE).  scores needs the d-contraction, which the PE cannot do in this
layout; it runs on VectorE as a fused multiply+reduce (tensor_tensor_reduce)
against a broadcast copy of q.  Context is read from HBM exactly once.

Per-core engine budget (4 batches): DMA ~38MB (bound), DVE ~78us,
ACT ~68us, PE ~50us, GpSimd small.
"""

import os
import sys

import numpy as np

_HERE = os.path.dirname(os.path.abspath(__file__))
if _HERE not in sys.path:
    sys.path.insert(0, _HERE)

import concourse.bacc as bacc
import concourse.bass as bass  # noqa: F401
import concourse.mybir as mybir
import concourse.tile as tile
from concourse.bass_isa import ReduceOp  # noqa: E402
from concourse.bass_utils import run_bass_kernel_spmd

try:
    from tile_patch import legalize_single_wait
except ImportError:
    # kernel.py must be self-contained: inline fallback of the walrus
    # single-sync-wait legalization (see tile_patch.py).
    _uid = [0]

    def legalize_single_wait(nc):
        for fn in nc.m.functions:
            for bb in fn.blocks:
                new_insts = []
                changed = False
                for inst in bb.instructions:
                    si = inst.sync_info
                    if si is not None and len(si.on_wait) > 1:
                        waits = list(si.on_wait)
                        for w in waits[:-1]:
                            _uid[0] += 1
                            nop = mybir.InstNoOp(
                                name=f"I-swl-{_uid[0]}",
                                engine=inst.engine,
                                sync_info=mybir.SyncInfo(on_wait=[w], on_update=[]),
                            )
                            new_insts.append(nop)
                        si.on_wait = waits[-1:]
                        changed = True
                    new_insts.append(inst)
                if changed:
                    bb.instructions = new_insts


NCORES = 8
B, L, D, D2 = 32, 2048, 1024, 512
BPC = B // NCORES            # batches per core
P = 128                      # partitions
NLC = L // P                 # 16 l-chunks
NDC = D // P                 # 8 d-chunks
NCC = 2 * D // P             # 16 c-chunks of combined
NOT = D2 // P                # 4 o-tiles

F32 = mybir.dt.float32
BF16 = mybir.dt.bfloat16
AF = mybir.ActivationFunctionType
ALU = mybir.AluOpType
AX = mybir.AxisListType


def build_bass():
    nc = bacc.Bacc("TRN2", target_bir_lowering=False)

    qt_d = nc.declare_dram_parameter("qt", [P, NDC * BPC], F32, isOutput=False)          # [p,(dc,b)]
    ctx_d = nc.declare_dram_parameter("ctx", [BPC, NLC // 2, P, 2, D], F32, isOutput=False)
    wint_d = nc.declare_dram_parameter("wint", [NDC, P, D], F32, isOutput=False)         # W_in.T
    woutt_d = nc.declare_dram_parameter("woutt", [NCC, P, D2], BF16, isOutput=False)    # W_out.T bf16
    w2b_d = nc.declare_dram_parameter("w2b", [BPC, P, NLC], F32, isOutput=False)         # ae*exp(-ab*delta)
    out_d = nc.declare_dram_parameter("out_o", [P, NOT * BPC], F32, isOutput=True)
    attn_d = nc.declare_dram_parameter("attn_o", [BPC, P, NLC], F32, isOutput=True)

    with tile.TileContext(nc) as tc:
        with (
            tc.tile_pool(name="wts", bufs=3) as kpool,
            tc.tile_pool(name="singles", bufs=1) as qpool,
            tc.tile_pool(name="cdata", bufs=5) as cpool,
            tc.tile_pool(name="rdata", bufs=24) as rpool,
            tc.tile_pool(name="cbdata", bufs=24) as cbpool,
            tc.tile_pool(name="qb", bufs=2) as qbpool,
            tc.tile_pool(name="rows", bufs=2) as rowpool,
            tc.tile_pool(name="smalls", bufs=6) as spool,
            tc.tile_pool(name="psA", bufs=1, space="PSUM") as psa,
            tc.tile_pool(name="psB", bufs=2, space="PSUM") as psb,
        ):
            # ---- W_in GEMM, both output layouts --------------------------
            qt = qpool.tile([P, NDC * BPC], F32)
            nc.sync.dma_start(out=qt[:], in_=qt_d[:])

            qrow_ps = psa.tile([BPC, D], F32)      # q as rows (for broadcast)
            for dc in range(NDC):
                w = kpool.tile([P, D], F32)
                nc.sync.dma_start(out=w[:], in_=wint_d[dc])
                lhs_q = qt[:, dc * BPC:(dc + 1) * BPC]
                nc.tensor.matmul(qrow_ps[:, 0:512], lhs_q, w[:, 0:512],
                                 start=(dc == 0), stop=(dc == NDC - 1))
                nc.tensor.matmul(qrow_ps[:, 512:1024], lhs_q, w[:, 512:1024],
                                 start=(dc == 0), stop=(dc == NDC - 1))

            q_rows = qpool.tile([BPC, D], F32)
            nc.vector.tensor_copy(q_rows[:], qrow_ps[:])

            w2b_sb = qpool.tile([P, BPC * NLC], F32)
            for b in range(BPC):
                nc.scalar.dma_start(out=w2b_sb[:, b * NLC:(b + 1) * NLC],
                                    in_=w2b_d[b])

            ones1 = qpool.tile([P, 1], F32)
            nc.vector.memset(ones1[:], 1.0)
            eye8 = qpool.tile([1, NDC, NDC], F32)
            nc.vector.memset(eye8[:], 0.0)
            for j in range(NDC):
                nc.vector.memset(eye8[:, j, j:j + 1], 1.0)
            comb_ps = psa.tile([P, NCC, BPC], F32)
            ops_ps = psa.tile([P, NOT * BPC], F32)

            # ---- per-batch main pipeline ---------------------------------
            for b in range(BPC):
                qrow_b = rowpool.tile([1, D], F32)
                nc.scalar.dma_start(out=qrow_b[:], in_=q_rows[b:b + 1, :])
                for j in range(NDC):
                    nc.tensor.matmul(
                        comb_ps[:, NDC:NCC, b],
                        qrow_b[:, j * P:(j + 1) * P], eye8[:, j, :],
                        start=(b == 0 and j == 0), stop=(j == NDC - 1),
                    )
                qb = qbpool.tile([P, D], F32)
                nc.gpsimd.partition_broadcast(qb[:], qrow_b[:])

                scores = spool.tile([P, NLC], F32)
                cb_tiles = []
                r_tiles = []
                for ii in range(NLC // 2):
                    cpt = cpool.tile([P, 2, D], F32)
                    nc.sync.dma_start(out=cpt[:], in_=ctx_d[b, ii])
                    for j in range(2):
                        i = ii * 2 + j
                        ct = cpt[:, j, :]
                        rt = rpool.tile([P, D], BF16)
                        cbt = cbpool.tile([P, D], BF16)
                        # balance relu/cast between ACT (1x) and DVE (2x):
                        # DVE budget beyond AMR ~ 9 units of 16+16
                        if i in (2, 7, 13):      # DVE relu
                            nc.vector.tensor_scalar_max(rt[:], ct[:], 0.0)
                        else:
                            nc.scalar.activation(rt[:], ct[:], AF.Relu)
                        if i in (0, 4, 6, 10, 12, 15):   # DVE cast
                            nc.vector.tensor_copy(cbt[:], ct[:])
                        else:
                            nc.scalar.copy(cbt[:], ct[:])
                        scr = qbpool.tile([P, D], F32, tag="amr_scr")
                        nc.vector.affine_mul_reduce(
                            scr[:], scores[:, i:i + 1], ct[:], qb[:], 1.0, 0.0,
                        )
                        # keep the PE HAM-warm: one cheap matmul per chunk
                        nc.tensor.matmul(ops_ps[0:1, 0:8], ones1[:], ct[:, 0:8],
                                         start=True, stop=True)
                        cb_tiles.append(cbt)
                        r_tiles.append(rt)

                # softmax over all 2048 scores (layout [128, 16])
                mx = spool.tile([P, 1], F32)
                nc.vector.tensor_reduce(mx[:], scores[:], axis=AX.X, op=ALU.max)
                nc.gpsimd.partition_all_reduce(mx[:], mx[:], P, ReduceOp.max)
                negmx = spool.tile([P, 1], F32)
                nc.vector.tensor_scalar_mul(negmx[:], mx[:], -1.0)
                ex = spool.tile([P, NLC], F32)
                nc.scalar.activation(ex[:], scores[:], AF.Exp, bias=negmx[:])
                sm = spool.tile([P, 1], F32)
                nc.vector.tensor_reduce(sm[:], ex[:], axis=AX.X, op=ALU.add)
                nc.gpsimd.partition_all_reduce(sm[:], sm[:], P, ReduceOp.add)
                rinv = spool.tile([P, 1], F32)
                nc.vector.reciprocal(rinv[:], sm[:])
                attn = spool.tile([P, NLC], F32)
                nc.vector.tensor_scalar_mul(attn[:], ex[:], rinv[:])
                nc.scalar.dma_start(out=attn_d[b], in_=attn[:])

                w2 = spool.tile([P, NLC], F32)
                nc.vector.tensor_mul(w2[:], attn[:],
                                     w2b_sb[:, b * NLC:(b + 1) * NLC])
                w2bf = spool.tile([P, NLC], BF16)
                nc.vector.tensor_copy(w2bf[:], w2[:])
                attn_bf = spool.tile([P, NLC], BF16)
                nc.vector.tensor_copy(attn_bf[:], attn[:])

                # mixed[d] = sum_l attn[l]*C[l,d] + w2[l]*relu(C)[l,d]
                # moving-operand GEMV: vector stationary (1-col LDW), C/R
                # tiles stream through the PE; out rows [1, 1024] in PSUM.
                mixed_ps = psb.tile([1, D], F32)
                for i in range(NLC):
                    for h in range(2):      # two 512-col banks
                        sl = slice(h * 512, (h + 1) * 512)
                        nc.tensor.matmul(
                            mixed_ps[:, sl],
                            attn_bf[:, i:i + 1],
                            cb_tiles[i][:, sl],
                            start=(i == 0), stop=False,
                        )
                        nc.tensor.matmul(
                            mixed_ps[:, sl],
                            w2bf[:, i:i + 1],
                            r_tiles[i][:, sl],
                            start=False, stop=(i == NLC - 1),
                        )
                # scatter row -> [128 d-part, 8 chunks] via K=1 matmuls
                mrow = rowpool.tile([1, D], F32)
                nc.vector.tensor_copy(mrow[:], mixed_ps[:])
                for j in range(NDC):
                    nc.tensor.matmul(
                        comb_ps[:, 0:NDC, b],
                        mrow[:, j * P:(j + 1) * P], eye8[:, j, :],
                        start=False, stop=(j == NDC - 1),
                    )

            # ---- W_out GEMM (batched) + tanh -----------------------------
            comb_bf = qpool.tile([P, NCC, BPC], BF16)
            nc.vector.tensor_copy(comb_bf[:], comb_ps[:])
            for cc in range(NCC):
                wo = kpool.tile([P, D2], BF16)
                nc.sync.dma_start(out=wo[:], in_=woutt_d[cc])
                for ot in range(NOT):
                    nc.tensor.matmul(
                        ops_ps[:, ot * BPC:(ot + 1) * BPC],
                        wo[:, ot * P:(ot + 1) * P],
                        comb_bf[:, cc, :],
                        start=(cc == 0 and ot == 0), stop=(cc == NCC - 1),
                    )
            out_sb = qpool.tile([P, NOT * BPC], F32)
            nc.scalar.activation(out_sb[:], ops_ps[:], AF.Tanh)
            nc.scalar.dma_start(out=out_d[:], in_=out_sb[:])

    nc.compile()
    legalize_single_wait(nc)
    return nc


_NC_CACHE = None


def _get_nc():
    global _NC_CACHE
    if _NC_CACHE is None:
        _NC_CACHE = build_bass()
    return _NC_CACHE


def _prep_in_maps(query, context, W_in, W_out, ae, ab):
    query = np.asarray(query, dtype=np.float32)
    context = np.asarray(context, dtype=np.float32)
    W_in = np.asarray(W_in, dtype=np.float32)
    W_out = np.asarray(W_out, dtype=np.float32)
    ae = np.asarray(ae, dtype=np.float32).reshape(B)
    ab = np.asarray(ab, dtype=np.float32).reshape(B)

    # Replicated weights, host-transposed for natural SBUF tiling.
    wint = np.ascontiguousarray(W_in.T.reshape(NDC, P, D))
    import ml_dtypes
    woutt = np.ascontiguousarray(
        W_out.T.reshape(NCC, P, D2).astype(ml_dtypes.bfloat16)
    )

    # delta in [p, c] layout: delta[c*128+p] = L-1-(c*128+p)
    lidx = (np.arange(NLC)[None, :] * P + np.arange(P)[:, None]).astype(np.float32)
    delta_pc = (L - 1) - lidx                                    # [128, 16]

    in_maps = []
    for m in range(NCORES):
        bs = slice(m * BPC, (m + 1) * BPC)
        q = query[bs, 0, :]                                       # [4, 1024]
        qt = np.ascontiguousarray(
            q.T.reshape(NDC, P, BPC).transpose(1, 0, 2).reshape(P, NDC * BPC)
        )
        ctx = np.ascontiguousarray(
            context[bs].reshape(BPC, NLC // 2, 2, P, D).transpose(0, 1, 3, 2, 4)
        )
        ae_m = ae[bs]
        ab_m = ab[bs]
        w2b = (ae_m[:, None, None]
               * np.exp(-ab_m[:, None, None] * delta_pc[None])).astype(np.float32)
        in_maps.append({
            "qt": qt,
            "ctx": ctx,
            "wint": wint,
            "woutt": woutt,
            "w2b": np.ascontiguousarray(w2b),
        })
    return in_maps


def _assemble(results):
    out = np.empty((B, 1, D2), dtype=np.float32)
    attn = np.empty((B, 1, L), dtype=np.float32)
    for m in range(NCORES):
        o = results[m]["out_o"]                       # [128, 16] = [p,(ot,b)]
        a = results[m]["attn_o"]                      # [4, 128, 16] = [b,p,c]
        out[m * BPC:(m + 1) * BPC, 0, :] = (
            o.reshape(P, NOT, BPC).transpose(2, 1, 0).reshape(BPC, D2)
        )
        attn[m * BPC:(m + 1) * BPC, 0, :] = (
            a.transpose(0, 2, 1).reshape(BPC, L)
        )
    return out, attn


def kernel(query, context, W_in, W_out, ae, ab):
    nc = _get_nc()
    in_maps = _prep_in_maps(query, context, W_in, W_out, ae, ab)
    res = run_bass_kernel_spmd(nc, in_maps, core_ids=list(range(NCORES)))
    return _assemble(res.results)


if __name__ == "__main__":
    # quick self-run against reference
    import reference

    inputs = {k: np.asarray(v) for k, v in reference.setup_inputs().items()}
    got_out, got_attn = kernel(**inputs)
    exp_out, exp_attn = reference.reference(**reference.setup_inputs())
    exp_out, exp_attn = np.asarray(exp_out), np.asarray(exp_attn)
    for name, g, e in (("out", got_out, exp_out), ("attn", got_attn, exp_attn)):
        rel = np.linalg.norm(g - e) / np.linalg.norm(e)
        print(f"{name}: rel_err={rel:.3e} max={np.abs(g - e).max():.3e}")
